# revision 10
# baseline (speedup 1.0000x reference)
"""Causal self-attention (B=4, T=2048, C=1024, H=16) on 8 TRN2 NeuronCores.

Sharding: core c handles batch b = c//2 and heads [8*(c%2), 8*(c%2)+8).
Each core computes the qkv projection for its 8 heads, flash-style causal
attention, and a partial output projection (its heads' slice of W_out rows).
Host sums the two partials per batch and adds the bias terms (v/out biases
are affine in the output because softmax rows sum to 1).

Device layouts (per core):
  xT [C, T]      x[b] transposed (host-side transpose)
  qT/kT [512,T]  bf16, heads stacked along partitions (64 rows/head)
  v  [T, 8*65]   bf16 natural, per head 65 cols: [v(64) | ones] -- the ones
                 column makes the AV matmul emit the softmax denominator.
  S^T blocks [128(s), 512(t)] in PSUM pairs -> one wide exp -> bf16 probs^T
  out^T accum [65, 512] in PSUM; normalize = reciprocal -> gpsimd
                 partition_broadcast -> one DVE multiply.
Projections run in float32r (tf32-class, 1 cyc/row at free dim >= 256);
attention S/AV matmuls in bf16 (uniform dtype avoids PE mode switches).
"""
from contextlib import ExitStack

import numpy as np
import concourse.bass as bass  # noqa: F401  (registers engines)
import concourse.mybir as mybir
import concourse.tile as tile
from concourse import bacc
from concourse.bass_utils import run_bass_kernel_spmd

# problem constants (hardcoded per contract)
B, T, C, H, D = 4, 2048, 1024, 16, 64
NCORES = 8
NH = H // 2          # heads per core = 8
QK = NH * D          # 512 qkv cols per core per q/k/v
SCALE = float(D) ** -0.5
P = 128
NKT = C // P         # 8 contraction tiles for the projections
NMQ = QK // P        # 4 row-tiles of qT/kT
NTT = T // P         # 16 t-blocks
NQC = T // 512       # 4 q-chunks
FP = mybir.dt.float32
FPR = mybir.dt.float32r
BF = mybir.dt.bfloat16
EXP = mybir.ActivationFunctionType.Exp

_NC_CACHE = {}
_LAST_IN_MAPS = None


def build_nc():
    if "nc" in _NC_CACHE:
        return _NC_CACHE["nc"]
    nc = bacc.Bacc(target_bir_lowering=False)
    xT = nc.declare_dram_parameter("xT", [C, T], BF, isOutput=False)
    Wq = nc.declare_dram_parameter("Wq", [NMQ, C, P], BF, isOutput=False)
    Wk = nc.declare_dram_parameter("Wk", [NMQ, C, P], BF, isOutput=False)
    Wv = nc.declare_dram_parameter("Wv", [C, QK], BF, isOutput=False)
    bq = nc.declare_dram_parameter("bq", [QK, 1], FP, isOutput=False)
    bk = nc.declare_dram_parameter("bk", [QK, 1], FP, isOutput=False)
    Wo = nc.declare_dram_parameter("Wo", [QK, C], BF, isOutput=False)
    tri = nc.declare_dram_parameter("tri", [P, P], BF, isOutput=False)
    y = nc.declare_dram_parameter("y", [T, C], FP, isOutput=True)

    with nc.allow_low_precision(reason="fp32r/bf16 attention"), \
         tile.TileContext(nc) as tc, \
         tc.tile_pool(name="persist", bufs=1) as pers:
        # ---- persistent tiles (one pool, one slot per tag)
        qT = [pers.tile([P, T], BF, name=f"qT{m}", tag=f"qT{m}")
              for m in range(NMQ)]
        kT = [pers.tile([P, T], BF, name=f"kT{m}", tag=f"kT{m}")
              for m in range(NMQ)]
        vsb = [pers.tile([P, NH * 65], BF, name=f"v{t}", tag=f"v{t}")
               for t in range(NTT)]
        trit = pers.tile([P, P], BF, name="trit", tag="trit")
        bqt = pers.tile([P, NMQ], FP, name="bqt", tag="bqt")
        bkt = pers.tile([P, NMQ], FP, name="bkt", tag="bkt")

        nc.sync.dma_start(trit, tri.ap())
        nc.sync.dma_start(bqt, bq.ap().rearrange("(m p) o -> p (m o)", p=P))
        nc.sync.dma_start(bkt, bk.ap().rearrange("(m p) o -> p (m o)", p=P))

        with ExitStack() as stk:
            pp = stk.enter_context(tc.tile_pool(name="psum", bufs=1, space="PSUM"))
            wp = stk.enter_context(tc.tile_pool(name="wpool", bufs=1))
            ep = stk.enter_context(tc.tile_pool(name="evict", bufs=1))

            # ================= phase 1: qkv projections =================
            with tc.tile_pool(name="xpool", bufs=1) as xp:
                xsb = []
                for k in range(NKT):
                    xt = xp.tile([P, T], BF, name=f"x{k}", tag=f"x{k}")
                    nc.sync.dma_start(xt, xT.ap()[k * P:(k + 1) * P, :])
                    xsb.append(xt)

                wvs = []
                for k in range(NKT):
                    wv = wp.tile([P, QK], BF, name=f"wv{k}", tag="wv", bufs=9)
                    nc.sync.dma_start(wv, Wv.ap()[k * P:(k + 1) * P, :])
                    wvs.append(wv)

                def v_pair(tp):
                    ps = pp.tile([P, 1024], FP, name=f"pv{tp}", tag="wide",
                                 bufs=3)
                    for i in range(2):
                        t = 2 * tp + i
                        for k in range(NKT):
                            nc.tensor.matmul(
                                ps[:, i * 512:(i + 1) * 512],
                                xsb[k][:, t * P:(t + 1) * P], wvs[k],
                                start=(k == 0), stop=(k == NKT - 1))
                    for i in range(2):
                        t = 2 * tp + i
                        vdst = vsb[t].rearrange("p (g w) -> p g w", w=65)
                        vsrc = ps[:, i * 512:(i + 1) * 512].rearrange(
                            "p (g w) -> p g w", w=64)
                        nc.vector.tensor_copy(vdst[:, :, 0:64], vsrc[:, :, :])
                        nc.vector.memset(vdst[:, :, 64:65], 1.0)

                # v tiles 0..3 first so attention's first AV groups unblock
                for tp in range(2):
                    v_pair(tp)

                for which, wdram, dst, bias in (
                    ("q", Wq, qT, bqt), ("k", Wk, kT, bkt),
                ):
                    for m in range(NMQ):
                        wts = []
                        for k in range(NKT):
                            wt = wp.tile([P, P], BF, name=f"w{which}{m}{k}",
                                         tag="w", bufs=10)
                            nc.sync.dma_start(
                                wt, wdram.ap()[m, k * P:(k + 1) * P, :])
                            wts.append(wt)
                        for half in range(2):
                            ps = pp.tile([P, 1024], FP, name=f"p{which}{m}{half}",
                                         tag="wide", bufs=3)
                            for sub in range(2):
                                c0 = half * 1024 + sub * 512
                                for k in range(NKT):
                                    nc.tensor.matmul(
                                        ps[:, sub * 512:(sub + 1) * 512],
                                        wts[k], xsb[k][:, c0:c0 + 512],
                                        start=(k == 0), stop=(k == NKT - 1))
                            nc.vector.tensor_scalar_add(
                                dst[m][:, half * 1024:(half + 1) * 1024], ps,
                                bias[:, m:m + 1])

                for tp in range(2, NTT // 2):
                    v_pair(tp)

            # late pool reuses the x tiles' freed space
            lp = stk.enter_context(tc.tile_pool(name="late", bufs=1))
            aT = [lp.tile([P, T], BF, name=f"aT{m}", tag=f"aT{m}")
                  for m in range(NMQ)]
            wot = [lp.tile([P, C], BF, name=f"wo{k}", tag=f"wo{k}")
                   for k in range(NMQ)]
            for k in range(NMQ):
                nc.sync.dma_start(wot[k], Wo.ap()[k * P:(k + 1) * P, :])

            # ============ phase 2 + 3 interleaved by q-chunk ============
            LOG = mybir.ActivationFunctionType.Log
            for qc in range(NQC):
                for h in range(NH):
                    ht, ho = h // 2, (h % 2) * 64
                    vlo = 65 * h
                    qap = qT[ht][ho:ho + 64, qc * 512:(qc + 1) * 512]
                    po = pp.tile([P, 512], FP, name=f"po{h}{qc}", tag="acc",
                                 bufs=2)
                    jmax = 4 * qc + 3
                    npairs = (jmax + 1) // 2
                    for pr in range(npairs):
                        ps = pp.tile([P, 1024], FP, name=f"ps{h}{qc}{pr}",
                                     tag="wide", bufs=3)
                        los = []
                        for i in range(2):
                            j = 2 * pr + i
                            r = j - 4 * qc
                            lo = 0 if r < 0 else min(128 * r, 256)
                            los.append(lo)
                            nc.tensor.matmul(
                                ps[:, i * 512 + lo:(i + 1) * 512],
                                kT[ht][ho:ho + 64, j * P:(j + 1) * P],
                                qap[:, lo:], start=True, stop=True)
                        es = ep.tile([P, 1024], BF, name=f"es{h}{qc}{pr}",
                                     tag="es", bufs=4)
                        nc.scalar.activation(es[:, los[0]:], ps[:, los[0]:],
                                             EXP, scale=SCALE)
                        for i in range(2):
                            j = 2 * pr + i
                            r = j - 4 * qc
                            if r >= 0:
                                if r == 3:
                                    nc.vector.memset(es[:, 768:896], 0.0)
                                tlo = i * 512 + 128 * r
                                nc.vector.tensor_mul(
                                    es[:, tlo:tlo + 128],
                                    es[:, tlo:tlo + 128], trit)
                        for i in range(2):
                            j = 2 * pr + i
                            lo = los[i]
                            nc.tensor.matmul(
                                po[0:65, lo:], vsb[j][:, vlo:vlo + 65],
                                es[:, i * 512 + lo:(i + 1) * 512],
                                start=(j == 0), stop=(j == jmax))
                    # normalize: 1/den = exp(-ln(den)) on ACT (same table
                    # set as the softmax exp), broadcast on gpsimd, one DVE
                    # multiply. Numerator eviction releases the psum bank.
                    onum = ep.tile([64, 512], BF, name=f"on{h}{qc}",
                                   tag="onum", bufs=3)
                    nc.vector.tensor_copy(onum, po[0:64, :])
                    lden = ep.tile([1, 512], FP, name=f"ld{h}{qc}", tag="lden",
                                   bufs=2)
                    nc.scalar.activation(lden, po[64:65, :], LOG)
                    rsep = ep.tile([1, 512], FP, name=f"rs{h}{qc}", tag="rsep",
                                   bufs=2)
                    nc.scalar.activation(rsep, lden, EXP, scale=-1.0)
                    bcs = ep.tile([64, 512], FP, name=f"bc{h}{qc}", tag="bcs",
                                  bufs=3)
                    nc.gpsimd.partition_broadcast(bcs, rsep)
                    nc.vector.tensor_mul(
                        aT[ht][ho:ho + 64, qc * 512:(qc + 1) * 512],
                        onum, bcs)

                # --- output projection for the t-tiles this qc completes
                for t in range(4 * qc, 4 * qc + 4):
                    ps = pp.tile([P, 1024], FP, name=f"py{t}", tag="wide",
                                 bufs=3)
                    for n in range(2):
                        for k in range(NMQ):
                            nc.tensor.matmul(
                                ps[:, n * 512:(n + 1) * 512],
                                aT[k][:, t * P:(t + 1) * P],
                                wot[k][:, n * 512:(n + 1) * 512],
                                start=(k == 0), stop=(k == NMQ - 1))
                    ye = ep.tile([P, 1024], FP, name=f"ye{t}", tag="ye",
                                 bufs=3)
                    nc.vector.tensor_copy(ye, ps)
                    nc.sync.dma_start(y.ap()[t * P:(t + 1) * P, :], ye)

    nc.compile()
    _NC_CACHE["nc"] = nc
    return nc


def kernel(x, W_qkv, b_qkv, W_out, b_out):
    global _LAST_IN_MAPS
    x = np.asarray(x, dtype=np.float32)
    W_qkv = np.asarray(W_qkv, dtype=np.float32)
    b_qkv = np.asarray(b_qkv, dtype=np.float32)
    W_out = np.asarray(W_out, dtype=np.float32)
    b_out = np.asarray(b_out, dtype=np.float32)
    import ml_dtypes

    bf16 = ml_dtypes.bfloat16
    tri = np.triu(np.ones((P, P), dtype=np.float32)).astype(bf16)
    in_maps = []
    for c in range(NCORES):
        b, hg = c // 2, c % 2
        cols = slice(hg * QK, (hg + 1) * QK)
        wq = W_qkv[:, 0 * C:1 * C][:, cols]
        wk = W_qkv[:, 1 * C:2 * C][:, cols]
        wv = W_qkv[:, 2 * C:3 * C][:, cols]
        in_maps.append({
            "xT": np.ascontiguousarray(x[b].T).astype(bf16),
            "Wq": np.ascontiguousarray(
                wq.reshape(C, NMQ, P).transpose(1, 0, 2)).astype(bf16),
            "Wk": np.ascontiguousarray(
                wk.reshape(C, NMQ, P).transpose(1, 0, 2)).astype(bf16),
            "Wv": np.ascontiguousarray(wv).astype(bf16),
            "bq": np.ascontiguousarray(b_qkv[0 * C:1 * C][cols, None]),
            "bk": np.ascontiguousarray(b_qkv[1 * C:2 * C][cols, None]),
            "Wo": np.ascontiguousarray(W_out[hg * QK:(hg + 1) * QK, :]).astype(bf16),
            "tri": tri,
        })
    _LAST_IN_MAPS = in_maps
    nc = build_nc()
    res = run_bass_kernel_spmd(nc, in_maps, core_ids=list(range(NCORES)))
    # v-bias and output bias are affine in the output: softmax rows sum to 1.
    extra = b_qkv[2 * C:3 * C] @ W_out + b_out
    out = np.empty((B, T, C), dtype=np.float32)
    for b in range(B):
        out[b] = res.results[2 * b]["y"] + res.results[2 * b + 1]["y"] + extra
    return out
